# revision 7
# baseline (speedup 1.0000x reference)
"""Causal self-attention (B=2, T=2048, D=2048, H=16, d=128) on 8 TRN2 NeuronCores.

Sharding: head-parallel compute, token-parallel output. Core c owns heads
{2c, 2c+1} for both batches: column-parallel QKV projection, per-head RoPE +
causal attention. The per-head attention outputs are exchanged with one
AllToAll per (batch, head), after which every core holds all 16 heads for its
own 256 rows and computes the full output projection locally. Host
concatenates the 8 contiguous row shards.

Schedule (v2): QKV projections for BOTH batches run before any collective so
launch skew between cores is absorbed by local compute, and no local DMA ever
queues behind collective descriptors in the HW DMA rings (post-collective
reads/writes go through gpsimd software DMA instead). The PE runs only the
essential matmuls: causal masking is a 0/1 multiply on DVE after the exp, and
softmax denominators are accumulated on DVE with a single f32r ones-matmul
per (head, 512-query) tile. Exp is applied to fused [128, 1024] score pairs
to halve ACT instruction overhead.

Host-prepped layouts (sharding/layout prep only — all math on device):
  xT      [2, D, T]    x transposed per batch (bf16)
  wqk     [D, 512]     qkv_w rows [q_h0,q_h1,k_h0,k_h1] transposed (bf16)
  wv      [D, 256]     qkv_w v rows transposed (bf16)
  wo      [D, D]       full out_w transposed (bf16)
  cosT/sinTs [128, T]  RoPE tables transposed; sinTs rows 0:64 negated
  masks   [4, 128, 512] multiplicative causal masks (1 / 0) for diag blocks
Matmuls run bf16 (1cyc/row); accumulation fp32 in PSUM; softmax
denominators fp32.
"""
import math
import numpy as np
import ml_dtypes
from contextlib import ExitStack

import concourse.bass as bass
import concourse.tile as tile
from concourse import bacc, mybir
from concourse.bass_utils import run_bass_kernel_spmd

F32 = mybir.dt.float32
F32R = mybir.dt.float32r
BF16 = mybir.dt.bfloat16
BF16_NP = ml_dtypes.bfloat16
AF = mybir.ActivationFunctionType
ALU = mybir.AluOpType

NC_ = 8           # cores
B, T, D = 2, 2048, 2048
H, HD = 16, 128   # heads, head_dim
HPC = H // NC_    # heads per core = 2
TS = 512          # t-super tile
NTS = T // TS     # 4
NCH = D // 128    # 16 contraction chunks
ROWS = T // NC_   # 256 own token rows per batch
SCALE = 1.0 / math.sqrt(HD)


def _build_program():
    nc = bacc.Bacc("TRN2", target_bir_lowering=False, debug=False, num_devices=NC_)

    xT_d = nc.dram_tensor("xT", [B, D, T], BF16, kind="ExternalInput")
    wqk_d = nc.dram_tensor("wqk", [D, 4 * 128], BF16, kind="ExternalInput")
    wv_d = nc.dram_tensor("wv", [D, 2 * 128], BF16, kind="ExternalInput")
    wo_d = nc.dram_tensor("wo", [D, D], BF16, kind="ExternalInput")
    cos_d = nc.dram_tensor("cosT", [128, T], BF16, kind="ExternalInput")
    sin_d = nc.dram_tensor("sinTs", [128, T], BF16, kind="ExternalInput")
    mask_d = nc.dram_tensor("masks", [4, 128, TS], BF16, kind="ExternalInput")
    ones_d = nc.dram_tensor("ones", [128, 128], F32, kind="ExternalInput")
    bqk_d = nc.dram_tensor("bqk", [128, 4], F32, kind="ExternalInput")
    bv_d = nc.dram_tensor("bv", [1, 2 * 128], BF16, kind="ExternalInput")
    bo_d = nc.dram_tensor("bo", [1, D], F32, kind="ExternalInput")
    out_d = nc.dram_tensor("out", [B, ROWS, D], F32, kind="ExternalOutput")

    with tile.TileContext(nc) as tc:
        with ExitStack() as ctx:
            consts = ctx.enter_context(tc.tile_pool(name="consts", bufs=1))
            qkv = ctx.enter_context(tc.tile_pool(name="qkv", bufs=1))
            dramp = ctx.enter_context(tc.tile_pool(name="dramp", bufs=1, space="DRAM"))

            wo_r = wo_d.ap().rearrange("(h p) o -> p h o", p=128)
            wo_t = consts.tile([128, H, D], BF16)
            cos_t = consts.tile([128, T], BF16)
            nc.scalar.dma_start(out=cos_t, in_=cos_d.ap())
            sin_t = consts.tile([128, T], BF16)
            nc.scalar.dma_start(out=sin_t, in_=sin_d.ap())
            mask_t = consts.tile([128, 4, TS], BF16)
            nc.scalar.dma_start(out=mask_t, in_=mask_d.ap().rearrange("m p n -> p m n"))
            ones_t = consts.tile([128, 128], F32)
            nc.scalar.dma_start(out=ones_t, in_=ones_d.ap())
            ones_b = consts.tile([128, 128], BF16)
            nc.scalar.activation(ones_b[:], ones_t[:], AF.Copy)
            bqk_t = consts.tile([128, 4], F32)
            nc.scalar.dma_start(out=bqk_t, in_=bqk_d.ap())
            bv_t = consts.tile([128, 2 * 128], BF16)
            nc.gpsimd.dma_start(out=bv_t, in_=bv_d.ap().partition_broadcast(128))
            bo_t = consts.tile([128, D], F32)
            nc.gpsimd.dma_start(out=bo_t, in_=bo_d.ap().partition_broadcast(128))

            q_t = {b: qkv.tile([128, HPC, T], BF16, tag=f"q{b}", name=f"q_t{b}")
                   for b in range(B)}
            k_t = {b: qkv.tile([128, HPC, T], BF16, tag=f"k{b}", name=f"k_t{b}")
                   for b in range(B)}
            v_t = {b: qkv.tile([128, NTS * 4, HPC, 128], BF16, tag=f"v{b}",
                               name=f"v_t{b}") for b in range(B)}

            def stage1(b, s1w, wqk_t, wv_t, wqk_r, wv_r, xp, qep, tmp, s1ps):
                for ts in range(NTS):
                    qkp = [s1ps.tile([128, TS], F32, tag=f"qkp{j}", name=f"qkp{j}")
                           for j in range(4)]
                    vp = [s1ps.tile([128, 2 * 128], F32, tag=f"vp{tb}",
                                    name=f"vp{tb}")[:] for tb in range(4)]
                    for ci in range(NCH):
                        if b == 0 and ts == 0:
                            nc.sync.dma_start(out=wqk_t[:, ci, :], in_=wqk_r[:, ci, :])
                            nc.sync.dma_start(out=wv_t[:, ci, :], in_=wv_r[:, ci, :])
                        elif b == 0 and ts == 1:
                            nc.sync.dma_start(out=wo_t[:, ci, :], in_=wo_r[:, ci, :])
                        xt = xp.tile([128, TS], BF16)
                        nc.sync.dma_start(
                            out=xt,
                            in_=xT_d.ap()[b, ci * 128:(ci + 1) * 128,
                                          ts * TS:(ts + 1) * TS],
                        )
                        st_, sp_ = ci == 0, ci == NCH - 1
                        for j in range(4):
                            nc.tensor.matmul(
                                qkp[j][:], wqk_t[:, ci, j * 128:(j + 1) * 128], xt[:],
                                start=st_, stop=sp_)
                        for tb in range(4):
                            nc.tensor.matmul(
                                vp[tb], xt[:, tb * 128:(tb + 1) * 128],
                                wv_t[:, ci, :], start=st_, stop=sp_)
                    # evict q/k to bf16 on ACT (plus a half-swapped copy for
                    # rotate_half); RoPE + bias fused on DVE. sinTs rows 0:64
                    # carry the rotate_half sign flip.
                    cs = cos_t[:, ts * TS:(ts + 1) * TS]
                    sn = sin_t[:, ts * TS:(ts + 1) * TS]
                    for j in range(4):
                        qe = qep.tile([128, TS], BF16, tag=f"qe{j}", name=f"qe{j}",
                                      bufs=2)
                        qs = qep.tile([128, TS], BF16, tag=f"qs{j}", name=f"qs{j}",
                                      bufs=2)
                        nc.scalar.activation(qe[:], qkp[j][:], AF.Copy)
                        nc.scalar.activation(qs[0:64, :], qe[64:128, :], AF.Copy)
                        nc.scalar.activation(qs[64:128, :], qe[0:64, :], AF.Copy)
                        t1 = tmp.tile([128, TS], BF16, tag="t1", bufs=2)
                        t2 = tmp.tile([128, TS], BF16, tag="t2", bufs=2)
                        nc.vector.tensor_mul(t1[:], qe[:], cs)
                        nc.vector.tensor_mul(t2[:], qs[:], sn)
                        dst = (q_t[b] if j < 2 else k_t[b])[:, j % 2,
                                                            ts * TS:(ts + 1) * TS]
                        nc.vector.scalar_tensor_tensor(
                            dst, t1[:], bqk_t[:, j:j + 1], t2[:], ALU.add, ALU.add)
                    for tb in range(4):
                        vdst = v_t[b][:, ts * 4 + tb, :, :]
                        nc.scalar.activation(
                            vdst, vp[tb].rearrange("p (h e) -> p h e", h=HPC),
                            AF.Copy)
                        nc.vector.tensor_add(
                            vdst, vdst,
                            bv_t[:].rearrange("p (h e) -> p h e", h=HPC))

            def attention(b, atps, prp, accp, bsp, aosp):
                # one AllToAll per (b, head); triggered as soon as that head's
                # normalized outputs are in DRAM. The per-tile epilogue
                # (denominator matmul, reciprocal, normalize, DRAM write) is
                # deferred until the next tile's first score pair so the PE
                # never waits on the DVE/gpsimd accumulation chain.
                a2a_in = [dramp.tile([NC_, 128, ROWS], BF16, tag=f"a2i{b}{hh}",
                                     name=f"a2i{b}{hh}") for hh in range(HPC)]
                a2a_out = [dramp.tile([NC_, 128, ROWS], BF16, tag=f"a2o{b}{hh}",
                                      name=f"a2o{b}{hh}") for hh in range(HPC)]

                def epilogue(pend):
                    op, acc, hh, ts = pend
                    sm = atps.tile([128, TS], F32, tag="sm", bufs=1)
                    nc.tensor.matmul(sm[:], ones_b[:], acc[:], start=True,
                                     stop=True)
                    bsb = bsp.tile([128, TS], F32, tag="bsb", bufs=2)
                    with nc.allow_low_precision(reason="softmax recip"):
                        nc.vector.reciprocal_approx_fast(bsb[:], sm[:])
                    aos = aosp.tile([128, TS], BF16, tag="aos", bufs=3)
                    nc.vector.tensor_mul(aos[:], op[:], bsb[:])
                    nc.gpsimd.dma_start(
                        out=a2a_in[hh][2 * ts:2 * ts + 2, :, :].transpose(
                            [1, 0, 2]),
                        in_=aos[:].rearrange("d (s q) -> d s q", s=2))

                pend = None
                for hh in range(HPC):
                    for ts in range(NTS):
                        op = atps.tile([128, TS], F32, tag="op", bufs=2)
                        acc = accp.tile([128, TS], BF16, tag="acc", bufs=2)
                        npair = 2 * (ts + 1)
                        prev = None
                        for p in range(npair):
                            st = atps.tile([128, 2, TS], F32, tag="st", bufs=2)
                            for h2 in range(2):
                                tk = 2 * p + h2
                                nc.tensor.matmul(
                                    st[:, h2, :],
                                    k_t[b][:, hh, tk * 128:(tk + 1) * 128],
                                    q_t[b][:, hh, ts * TS:(ts + 1) * TS],
                                    start=True, stop=True)
                            if p == 1 and pend is not None:
                                epilogue(pend)
                                pend = None
                            pr = prp.tile([128, 2, TS], BF16, tag="pr", bufs=3)
                            nc.scalar.activation(pr[:], st[:], AF.Exp, scale=SCALE)
                            if p >= 2 * ts:  # diagonal pair: zero masked scores
                                mi = p - 2 * ts
                                nc.vector.tensor_mul(
                                    pr[:], pr[:], mask_t[:, 2 * mi:2 * mi + 2, :])
                            ps = bsp.tile([128, TS], BF16, tag="ps", bufs=2)
                            nc.vector.tensor_add(ps[:], pr[:, 0, :], pr[:, 1, :])
                            if p == 0:
                                nc.gpsimd.tensor_scalar_add(acc[:], ps[:], 0.0)
                            else:
                                nc.gpsimd.tensor_add(acc[:], acc[:], ps[:])
                            if prev is not None:
                                pp, ppr = prev
                                for h2 in range(2):
                                    nc.tensor.matmul(
                                        op[:], v_t[b][:, 2 * pp + h2, hh, :],
                                        ppr[:, h2, :],
                                        start=(pp == 0 and h2 == 0), stop=False)
                            prev = (p, pr)
                        pp, ppr = prev
                        for h2 in range(2):
                            nc.tensor.matmul(
                                op[:], v_t[b][:, 2 * pp + h2, hh, :], ppr[:, h2, :],
                                start=(pp == 0 and h2 == 0), stop=(h2 == 1))
                        pend = (op, acc, hh, ts)
                    # flush before the collective: it needs every tile's aos
                    epilogue(pend)
                    pend = None
                    nc.gpsimd.collective_compute(
                        "AllToAll", mybir.AluOpType.bypass,
                        replica_groups=[list(range(NC_))],
                        ins=[a2a_in[hh].opt()], outs=[a2a_out[hh].opt()])
                return a2a_out

            def outproj(b, a2a_out, aogp, yop, yps):
                # a2a_out[hh][src, d, q] == head (2*src+hh) for my ROWS of batch b
                aoG = [aogp.tile([128, NC_, ROWS], BF16, tag=f"aoG{hh}",
                                 name=f"aoG{hh}") for hh in range(HPC)]
                for hh in range(HPC):
                    nc.sync.dma_start(
                        out=aoG[hh],
                        in_=a2a_out[hh][:, :, :].rearrange("s d q -> d s q"))
                yp = {(tb, nb): yps.tile([128, TS], F32, tag=f"yp{tb}{nb}",
                                         name=f"yp{tb}{nb}")
                      for tb in range(2) for nb in range(4)}
                # hh-outer so the last batch's hh=0 accumulation starts as soon
                # as its first AllToAll lands; aoG slices stay stationary
                # across the four nb streams.
                for hh in range(HPC):
                    for s in range(NC_):
                        for tb in range(2):
                            stat = aoG[hh][:, s, tb * 128:(tb + 1) * 128]
                            for nb in range(4):
                                nc.tensor.matmul(
                                    yp[(tb, nb)][:], stat,
                                    wo_t[:, 2 * s + hh, nb * TS:(nb + 1) * TS],
                                    start=(hh == 0 and s == 0),
                                    stop=(hh == 1 and s == NC_ - 1))
                for tb in range(2):
                    for nb in range(4):
                        yo = yop.tile([128, TS], F32, tag="yo", bufs=3)
                        nc.vector.tensor_add(yo[:], yp[(tb, nb)][:],
                                             bo_t[:, nb * TS:(nb + 1) * TS])
                        nc.scalar.dma_start(
                            out=out_d.ap()[b, tb * 128:(tb + 1) * 128,
                                           nb * TS:(nb + 1) * TS],
                            in_=yo[:])

            # ---- phase 1: QKV projections for both batches (no collectives) --
            with tc.tile_pool(name="s1w", bufs=1) as s1w, \
                    tc.tile_pool(name="xp", bufs=8) as xp, \
                    tc.tile_pool(name="qep", bufs=1) as qep, \
                    tc.tile_pool(name="tmp", bufs=1) as tmp, \
                    tc.tile_pool(name="s1ps", bufs=1, space="PSUM") as s1ps:
                wqk_t = s1w.tile([128, NCH, 4 * 128], BF16)
                wv_t = s1w.tile([128, NCH, 2 * 128], BF16)
                wqk_r = wqk_d.ap().rearrange("(c p) e -> p c e", p=128)
                wv_r = wv_d.ap().rearrange("(c p) e -> p c e", p=128)
                for b in range(B):
                    stage1(b, s1w, wqk_t, wv_t, wqk_r, wv_r, xp, qep, tmp, s1ps)

            # ---- phase 2: attention + exchanges --------------------------
            a2a_outs = {}
            with tc.tile_pool(name="atps", bufs=1, space="PSUM") as atps, \
                    tc.tile_pool(name="prp", bufs=1) as prp, \
                    tc.tile_pool(name="accp", bufs=1) as accp, \
                    tc.tile_pool(name="bsp", bufs=1) as bsp, \
                    tc.tile_pool(name="aosp", bufs=1) as aosp:
                for b in range(B):
                    a2a_outs[b] = attention(b, atps, prp, accp, bsp, aosp)

            # ---- phase 3: output projections -----------------------------
            with tc.tile_pool(name="yps", bufs=1, space="PSUM") as yps, \
                    tc.tile_pool(name="aogp", bufs=1) as aogp, \
                    tc.tile_pool(name="yop", bufs=1) as yop:
                for b in range(B):
                    outproj(b, a2a_outs[b], aogp, yop, yps)

    nc.compile()
    return nc


_NC_CACHE = None


def _get_program():
    global _NC_CACHE
    if _NC_CACHE is None:
        _NC_CACHE = _build_program()
    return _NC_CACHE


def make_in_maps(x, rope_cos, rope_sin, qkv_w, qkv_b, out_w, out_b):
    x = np.asarray(x, dtype=np.float32)
    qkv_w = np.asarray(qkv_w, dtype=np.float32)
    qkv_b = np.asarray(qkv_b, dtype=np.float32)
    out_w = np.asarray(out_w, dtype=np.float32)
    out_b = np.asarray(out_b, dtype=np.float32)

    xT = np.ascontiguousarray(x.transpose(0, 2, 1)).astype(BF16_NP)  # [B, D, T]
    cosT = np.ascontiguousarray(np.asarray(rope_cos, np.float32)[0, 0].T).astype(BF16_NP)
    sinTs = np.ascontiguousarray(np.asarray(rope_sin, np.float32)[0, 0].T).copy()
    sinTs[0:64, :] *= -1.0
    sinTs = sinTs.astype(BF16_NP)

    tk_idx = np.arange(128)[:, None]
    tq_idx = np.arange(TS)[None, :]
    masks = np.stack(
        [np.where(mi * 128 + tk_idx <= tq_idx, 1.0, 0.0) for mi in range(4)]
    ).astype(BF16_NP)                                           # [4, 128, TS]
    ones = np.ones((128, 128), np.float32)
    wo = np.ascontiguousarray(out_w.T).astype(BF16_NP)          # [D, D]
    bo = out_b.reshape(1, D)

    in_maps = []
    for c in range(NC_):
        h0 = HPC * c
        qr = qkv_w[h0 * 128:(h0 + HPC) * 128]                  # [256, D]
        kr = qkv_w[D + h0 * 128:D + (h0 + HPC) * 128]
        vr = qkv_w[2 * D + h0 * 128:2 * D + (h0 + HPC) * 128]
        wqk = np.ascontiguousarray(np.concatenate([qr, kr], 0).T).astype(BF16_NP)
        wv = np.ascontiguousarray(vr.T).astype(BF16_NP)        # [D, 256]
        bqk = np.stack(
            [qkv_b[h0 * 128:(h0 + 1) * 128],
             qkv_b[(h0 + 1) * 128:(h0 + 2) * 128],
             qkv_b[D + h0 * 128:D + (h0 + 1) * 128],
             qkv_b[D + (h0 + 1) * 128:D + (h0 + 2) * 128]], axis=1)  # [128, 4]
        bv = qkv_b[2 * D + h0 * 128:2 * D + (h0 + HPC) * 128].reshape(1, 256)
        in_maps.append({
            "xT": xT, "wqk": wqk, "wv": wv, "wo": wo,
            "cosT": cosT, "sinTs": sinTs, "masks": masks, "ones": ones,
            "bqk": np.ascontiguousarray(bqk),
            "bv": np.ascontiguousarray(bv).astype(BF16_NP),
            "bo": bo,
        })
    return in_maps


def assemble(results):
    y = np.empty((B, T, D), dtype=np.float32)
    for c in range(NC_):
        y[:, c * ROWS:(c + 1) * ROWS, :] = results[c]["out"]
    return y


def run(inputs, trace=False, trace_cores=None):
    nc = _get_program()
    in_maps = make_in_maps(**inputs)
    res = run_bass_kernel_spmd(
        nc, in_maps, list(range(NC_)), trace=trace,
        trace_cores=trace_cores if trace else None)
    return assemble(res.results), res


def kernel(**inputs) -> np.ndarray:
    y, _ = run(inputs, trace=False)
    return y


# revision 8
# speedup vs baseline: 1.2842x; 1.2842x over previous
"""Causal self-attention (B=2, T=2048, D=2048, H=16, d=128) on 8 TRN2 NeuronCores.

Sharding: head-parallel compute, token-parallel output. Core c owns heads
{2c, 2c+1} for both batches: column-parallel QKV projection, per-head RoPE +
causal attention. The per-head attention outputs are exchanged with one
AllToAll per (batch, head), after which every core holds all 16 heads for its
own 256 rows and computes the full output projection locally. Host
concatenates the 8 contiguous row shards.

Schedule (v2): QKV projections for BOTH batches run before any collective so
launch skew between cores is absorbed by local compute, and no local DMA ever
queues behind collective descriptors in the HW DMA rings (post-collective
reads/writes go through gpsimd software DMA instead). The PE runs only the
essential matmuls: causal masking is a 0/1 multiply on DVE after the exp, and
softmax denominators are accumulated on DVE with a single f32r ones-matmul
per (head, 512-query) tile. Exp is applied to fused [128, 1024] score pairs
to halve ACT instruction overhead.

Host-prepped layouts (sharding/layout prep only — all math on device):
  xT      [2, D, T]    x transposed per batch (bf16)
  wqk     [D, 512]     qkv_w rows [q_h0,q_h1,k_h0,k_h1] transposed (bf16)
  wv      [D, 256]     qkv_w v rows transposed (bf16)
  wo      [D, D]       full out_w transposed (bf16)
  cosT/sinTs [128, T]  RoPE tables transposed; sinTs rows 0:64 negated
  masks   [4, 128, 512] multiplicative causal masks (1 / 0) for diag blocks
Matmuls run bf16 (1cyc/row); accumulation fp32 in PSUM; softmax
denominators fp32.
"""
import math
import numpy as np
import ml_dtypes
from contextlib import ExitStack

import concourse.bass as bass
import concourse.tile as tile
from concourse import bacc, mybir
from concourse.bass_utils import run_bass_kernel_spmd

F32 = mybir.dt.float32
F32R = mybir.dt.float32r
BF16 = mybir.dt.bfloat16
BF16_NP = ml_dtypes.bfloat16
AF = mybir.ActivationFunctionType
ALU = mybir.AluOpType

NC_ = 8           # cores
B, T, D = 2, 2048, 2048
H, HD = 16, 128   # heads, head_dim
HPC = H // NC_    # heads per core = 2
TS = 512          # t-super tile
NTS = T // TS     # 4
NCH = D // 128    # 16 contraction chunks
ROWS = T // NC_   # 256 own token rows per batch
SCALE = 1.0 / math.sqrt(HD)


def _build_program():
    nc = bacc.Bacc("TRN2", target_bir_lowering=False, debug=False, num_devices=NC_)

    xT_d = nc.dram_tensor("xT", [B, D, T], BF16, kind="ExternalInput")
    wqk_d = nc.dram_tensor("wqk", [D, 4 * 128], BF16, kind="ExternalInput")
    wv_d = nc.dram_tensor("wv", [D, 2 * 128], BF16, kind="ExternalInput")
    wo_d = nc.dram_tensor("wo", [D, D], BF16, kind="ExternalInput")
    cos_d = nc.dram_tensor("cosT", [128, T], BF16, kind="ExternalInput")
    sin_d = nc.dram_tensor("sinTs", [128, T], BF16, kind="ExternalInput")
    mask_d = nc.dram_tensor("masks", [4, 128, TS], BF16, kind="ExternalInput")
    ones_d = nc.dram_tensor("ones", [128, 128], F32, kind="ExternalInput")
    bqk_d = nc.dram_tensor("bqk", [128, 4], F32, kind="ExternalInput")
    bv_d = nc.dram_tensor("bv", [1, 2 * 128], BF16, kind="ExternalInput")
    bo_d = nc.dram_tensor("bo", [1, D], F32, kind="ExternalInput")
    out_d = nc.dram_tensor("out", [B, ROWS, D], F32, kind="ExternalOutput")

    with tile.TileContext(nc) as tc:
        with ExitStack() as ctx:
            consts = ctx.enter_context(tc.tile_pool(name="consts", bufs=1))
            qkv = ctx.enter_context(tc.tile_pool(name="qkv", bufs=1))
            dramp = ctx.enter_context(tc.tile_pool(name="dramp", bufs=1, space="DRAM"))

            wo_r = wo_d.ap().rearrange("(h p) o -> p h o", p=128)
            wo_t = consts.tile([128, H, D], BF16)
            cos_t = consts.tile([128, T], BF16)
            nc.scalar.dma_start(out=cos_t, in_=cos_d.ap())
            sin_t = consts.tile([128, T], BF16)
            nc.scalar.dma_start(out=sin_t, in_=sin_d.ap())
            mask_t = consts.tile([128, 4, TS], BF16)
            nc.scalar.dma_start(out=mask_t, in_=mask_d.ap().rearrange("m p n -> p m n"))
            ones_t = consts.tile([128, 128], F32)
            nc.scalar.dma_start(out=ones_t, in_=ones_d.ap())
            ones_b = consts.tile([128, 128], BF16)
            nc.scalar.activation(ones_b[:], ones_t[:], AF.Copy)
            bqk_t = consts.tile([128, 4], F32)
            nc.scalar.dma_start(out=bqk_t, in_=bqk_d.ap())
            bv_t = consts.tile([128, 2 * 128], BF16)
            nc.gpsimd.dma_start(out=bv_t, in_=bv_d.ap().partition_broadcast(128))
            bo_t = consts.tile([128, D], F32)
            nc.gpsimd.dma_start(out=bo_t, in_=bo_d.ap().partition_broadcast(128))

            q_t = {b: qkv.tile([128, HPC, T], BF16, tag=f"q{b}", name=f"q_t{b}")
                   for b in range(B)}
            k_t = {b: qkv.tile([128, HPC, T], BF16, tag=f"k{b}", name=f"k_t{b}")
                   for b in range(B)}
            v_t = {b: qkv.tile([128, NTS * 4, HPC, 128], BF16, tag=f"v{b}",
                               name=f"v_t{b}") for b in range(B)}

            def stage1(b, s1w, wqk_t, wv_t, wqk_r, wv_r, xp, qep, tmp, s1ps):
                for ts in range(NTS):
                    qkp = [s1ps.tile([128, TS], F32, tag=f"qkp{j}", name=f"qkp{j}")
                           for j in range(4)]
                    vp = [s1ps.tile([128, 2 * 128], F32, tag=f"vp{tb}",
                                    name=f"vp{tb}")[:] for tb in range(4)]
                    for ci in range(NCH):
                        if b == 0 and ts == 0:
                            nc.sync.dma_start(out=wqk_t[:, ci, :], in_=wqk_r[:, ci, :])
                            nc.sync.dma_start(out=wv_t[:, ci, :], in_=wv_r[:, ci, :])
                        elif b == 0 and ts == 1:
                            nc.sync.dma_start(out=wo_t[:, ci, :], in_=wo_r[:, ci, :])
                        xt = xp.tile([128, TS], BF16)
                        nc.sync.dma_start(
                            out=xt,
                            in_=xT_d.ap()[b, ci * 128:(ci + 1) * 128,
                                          ts * TS:(ts + 1) * TS],
                        )
                        st_, sp_ = ci == 0, ci == NCH - 1
                        for j in range(4):
                            nc.tensor.matmul(
                                qkp[j][:], wqk_t[:, ci, j * 128:(j + 1) * 128], xt[:],
                                start=st_, stop=sp_)
                        for tb in range(4):
                            nc.tensor.matmul(
                                vp[tb], xt[:, tb * 128:(tb + 1) * 128],
                                wv_t[:, ci, :], start=st_, stop=sp_)
                    # evict q/k to bf16 on ACT (plus a half-swapped copy for
                    # rotate_half); RoPE + bias fused on DVE. sinTs rows 0:64
                    # carry the rotate_half sign flip.
                    cs = cos_t[:, ts * TS:(ts + 1) * TS]
                    sn = sin_t[:, ts * TS:(ts + 1) * TS]
                    for j in range(4):
                        qe = qep.tile([128, TS], BF16, tag=f"qe{j}", name=f"qe{j}",
                                      bufs=2)
                        qs = qep.tile([128, TS], BF16, tag=f"qs{j}", name=f"qs{j}",
                                      bufs=2)
                        nc.scalar.activation(qe[:], qkp[j][:], AF.Copy)
                        nc.scalar.activation(qs[0:64, :], qe[64:128, :], AF.Copy)
                        nc.scalar.activation(qs[64:128, :], qe[0:64, :], AF.Copy)
                        t1 = tmp.tile([128, TS], BF16, tag="t1", bufs=2)
                        t2 = tmp.tile([128, TS], BF16, tag="t2", bufs=2)
                        nc.vector.tensor_mul(t1[:], qe[:], cs)
                        nc.vector.tensor_mul(t2[:], qs[:], sn)
                        dst = (q_t[b] if j < 2 else k_t[b])[:, j % 2,
                                                            ts * TS:(ts + 1) * TS]
                        nc.vector.scalar_tensor_tensor(
                            dst, t1[:], bqk_t[:, j:j + 1], t2[:], ALU.add, ALU.add)
                    for tb in range(4):
                        vdst = v_t[b][:, ts * 4 + tb, :, :]
                        nc.scalar.activation(
                            vdst, vp[tb].rearrange("p (h e) -> p h e", h=HPC),
                            AF.Copy)
                        nc.vector.tensor_add(
                            vdst, vdst,
                            bv_t[:].rearrange("p (h e) -> p h e", h=HPC))

            def attention(b, atps, prp, accp, bsp, aosp):
                # one AllToAll per (b, head); triggered as soon as that head's
                # normalized outputs are in DRAM. The per-tile epilogue
                # (denominator matmul, reciprocal, normalize, DRAM write) is
                # deferred until the next tile's first score pair so the PE
                # never waits on the DVE/gpsimd accumulation chain.
                a2a_in = [dramp.tile([NC_, 128, ROWS], BF16, tag=f"a2i{b}{hh}",
                                     name=f"a2i{b}{hh}") for hh in range(HPC)]
                a2a_out = [dramp.tile([NC_, 128, ROWS], BF16, tag=f"a2o{b}{hh}",
                                      name=f"a2o{b}{hh}") for hh in range(HPC)]

                def epilogue(pend):
                    op, acc, hh, ts = pend
                    sm = atps.tile([128, TS], F32, tag="sm", bufs=1)
                    nc.tensor.matmul(sm[:], ones_b[:], acc[:], start=True,
                                     stop=True)
                    bsb = bsp.tile([128, TS], F32, tag="bsb", bufs=2)
                    with nc.allow_low_precision(reason="softmax recip"):
                        nc.vector.reciprocal_approx_fast(bsb[:], sm[:])
                    aos = aosp.tile([128, TS], BF16, tag="aos", bufs=3)
                    nc.vector.tensor_mul(aos[:], op[:], bsb[:])
                    nc.gpsimd.dma_start(
                        out=a2a_in[hh][2 * ts:2 * ts + 2, :, :].transpose(
                            [1, 0, 2]),
                        in_=aos[:].rearrange("d (s q) -> d s q", s=2))

                pend = None
                for hh in range(HPC):
                    for ts in range(NTS):
                        op = atps.tile([128, TS], F32, tag="op", bufs=2)
                        acc = accp.tile([128, TS], BF16, tag="acc", bufs=2)
                        npair = 2 * (ts + 1)
                        prev = None
                        for p in range(npair):
                            st = atps.tile([128, 2, TS], F32, tag="st", bufs=2)
                            for h2 in range(2):
                                tk = 2 * p + h2
                                nc.tensor.matmul(
                                    st[:, h2, :],
                                    k_t[b][:, hh, tk * 128:(tk + 1) * 128],
                                    q_t[b][:, hh, ts * TS:(ts + 1) * TS],
                                    start=True, stop=True)
                            if p == 1 and pend is not None:
                                epilogue(pend)
                                pend = None
                            pr = prp.tile([128, 2, TS], BF16, tag="pr", bufs=3)
                            nc.scalar.activation(pr[:], st[:], AF.Exp, scale=SCALE)
                            if p >= 2 * ts:  # diagonal pair: zero masked scores
                                mi = p - 2 * ts
                                nc.vector.tensor_mul(
                                    pr[:], pr[:], mask_t[:, 2 * mi:2 * mi + 2, :])
                            ps = bsp.tile([128, TS], BF16, tag="ps", bufs=2)
                            nc.vector.tensor_add(ps[:], pr[:, 0, :], pr[:, 1, :])
                            if p == 0:
                                nc.vector.tensor_scalar_add(acc[:], ps[:], 0.0)
                            else:
                                nc.vector.tensor_add(acc[:], acc[:], ps[:])
                            if prev is not None:
                                pp, ppr = prev
                                for h2 in range(2):
                                    nc.tensor.matmul(
                                        op[:], v_t[b][:, 2 * pp + h2, hh, :],
                                        ppr[:, h2, :],
                                        start=(pp == 0 and h2 == 0), stop=False)
                            prev = (p, pr)
                        pp, ppr = prev
                        for h2 in range(2):
                            nc.tensor.matmul(
                                op[:], v_t[b][:, 2 * pp + h2, hh, :], ppr[:, h2, :],
                                start=(pp == 0 and h2 == 0), stop=(h2 == 1))
                        pend = (op, acc, hh, ts)
                    # flush before the collective: it needs every tile's aos
                    epilogue(pend)
                    pend = None
                    nc.gpsimd.collective_compute(
                        "AllToAll", mybir.AluOpType.bypass,
                        replica_groups=[list(range(NC_))],
                        ins=[a2a_in[hh].opt()], outs=[a2a_out[hh].opt()])
                return a2a_out

            def outproj(b, a2a_out, aogp, yop, yps):
                # a2a_out[hh][src, d, q] == head (2*src+hh) for my ROWS of batch b
                aoG = [aogp.tile([128, NC_, ROWS], BF16, tag=f"aoG{hh}",
                                 name=f"aoG{hh}") for hh in range(HPC)]
                for hh in range(HPC):
                    nc.sync.dma_start(
                        out=aoG[hh],
                        in_=a2a_out[hh][:, :, :].rearrange("s d q -> d s q"))
                yp = {(tb, nb): yps.tile([128, TS], F32, tag=f"yp{tb}{nb}",
                                         name=f"yp{tb}{nb}")
                      for tb in range(2) for nb in range(4)}
                # hh-outer so the last batch's hh=0 accumulation starts as soon
                # as its first AllToAll lands; aoG slices stay stationary
                # across the four nb streams.
                for hh in range(HPC):
                    for s in range(NC_):
                        for tb in range(2):
                            stat = aoG[hh][:, s, tb * 128:(tb + 1) * 128]
                            for nb in range(4):
                                nc.tensor.matmul(
                                    yp[(tb, nb)][:], stat,
                                    wo_t[:, 2 * s + hh, nb * TS:(nb + 1) * TS],
                                    start=(hh == 0 and s == 0),
                                    stop=(hh == 1 and s == NC_ - 1))
                for tb in range(2):
                    for nb in range(4):
                        yo = yop.tile([128, TS], F32, tag="yo", bufs=3)
                        nc.vector.tensor_add(yo[:], yp[(tb, nb)][:],
                                             bo_t[:, nb * TS:(nb + 1) * TS])
                        nc.scalar.dma_start(
                            out=out_d.ap()[b, tb * 128:(tb + 1) * 128,
                                           nb * TS:(nb + 1) * TS],
                            in_=yo[:])

            # ---- phase 1: QKV projections for both batches (no collectives) --
            with tc.tile_pool(name="s1w", bufs=1) as s1w, \
                    tc.tile_pool(name="xp", bufs=8) as xp, \
                    tc.tile_pool(name="qep", bufs=1) as qep, \
                    tc.tile_pool(name="tmp", bufs=1) as tmp, \
                    tc.tile_pool(name="s1ps", bufs=1, space="PSUM") as s1ps:
                wqk_t = s1w.tile([128, NCH, 4 * 128], BF16)
                wv_t = s1w.tile([128, NCH, 2 * 128], BF16)
                wqk_r = wqk_d.ap().rearrange("(c p) e -> p c e", p=128)
                wv_r = wv_d.ap().rearrange("(c p) e -> p c e", p=128)
                for b in range(B):
                    stage1(b, s1w, wqk_t, wv_t, wqk_r, wv_r, xp, qep, tmp, s1ps)

            # ---- phase 2: attention + exchanges --------------------------
            a2a_outs = {}
            with tc.tile_pool(name="atps", bufs=1, space="PSUM") as atps, \
                    tc.tile_pool(name="prp", bufs=1) as prp, \
                    tc.tile_pool(name="accp", bufs=1) as accp, \
                    tc.tile_pool(name="bsp", bufs=1) as bsp, \
                    tc.tile_pool(name="aosp", bufs=1) as aosp:
                for b in range(B):
                    a2a_outs[b] = attention(b, atps, prp, accp, bsp, aosp)

            # ---- phase 3: output projections -----------------------------
            with tc.tile_pool(name="yps", bufs=1, space="PSUM") as yps, \
                    tc.tile_pool(name="aogp", bufs=1) as aogp, \
                    tc.tile_pool(name="yop", bufs=1) as yop:
                for b in range(B):
                    outproj(b, a2a_outs[b], aogp, yop, yps)

    nc.compile()
    return nc


_NC_CACHE = None


def _get_program():
    global _NC_CACHE
    if _NC_CACHE is None:
        _NC_CACHE = _build_program()
    return _NC_CACHE


def make_in_maps(x, rope_cos, rope_sin, qkv_w, qkv_b, out_w, out_b):
    x = np.asarray(x, dtype=np.float32)
    qkv_w = np.asarray(qkv_w, dtype=np.float32)
    qkv_b = np.asarray(qkv_b, dtype=np.float32)
    out_w = np.asarray(out_w, dtype=np.float32)
    out_b = np.asarray(out_b, dtype=np.float32)

    xT = np.ascontiguousarray(x.transpose(0, 2, 1)).astype(BF16_NP)  # [B, D, T]
    cosT = np.ascontiguousarray(np.asarray(rope_cos, np.float32)[0, 0].T).astype(BF16_NP)
    sinTs = np.ascontiguousarray(np.asarray(rope_sin, np.float32)[0, 0].T).copy()
    sinTs[0:64, :] *= -1.0
    sinTs = sinTs.astype(BF16_NP)

    tk_idx = np.arange(128)[:, None]
    tq_idx = np.arange(TS)[None, :]
    masks = np.stack(
        [np.where(mi * 128 + tk_idx <= tq_idx, 1.0, 0.0) for mi in range(4)]
    ).astype(BF16_NP)                                           # [4, 128, TS]
    ones = np.ones((128, 128), np.float32)
    wo = np.ascontiguousarray(out_w.T).astype(BF16_NP)          # [D, D]
    bo = out_b.reshape(1, D)

    in_maps = []
    for c in range(NC_):
        h0 = HPC * c
        qr = qkv_w[h0 * 128:(h0 + HPC) * 128]                  # [256, D]
        kr = qkv_w[D + h0 * 128:D + (h0 + HPC) * 128]
        vr = qkv_w[2 * D + h0 * 128:2 * D + (h0 + HPC) * 128]
        wqk = np.ascontiguousarray(np.concatenate([qr, kr], 0).T).astype(BF16_NP)
        wv = np.ascontiguousarray(vr.T).astype(BF16_NP)        # [D, 256]
        bqk = np.stack(
            [qkv_b[h0 * 128:(h0 + 1) * 128],
             qkv_b[(h0 + 1) * 128:(h0 + 2) * 128],
             qkv_b[D + h0 * 128:D + (h0 + 1) * 128],
             qkv_b[D + (h0 + 1) * 128:D + (h0 + 2) * 128]], axis=1)  # [128, 4]
        bv = qkv_b[2 * D + h0 * 128:2 * D + (h0 + HPC) * 128].reshape(1, 256)
        in_maps.append({
            "xT": xT, "wqk": wqk, "wv": wv, "wo": wo,
            "cosT": cosT, "sinTs": sinTs, "masks": masks, "ones": ones,
            "bqk": np.ascontiguousarray(bqk),
            "bv": np.ascontiguousarray(bv).astype(BF16_NP),
            "bo": bo,
        })
    return in_maps


def assemble(results):
    y = np.empty((B, T, D), dtype=np.float32)
    for c in range(NC_):
        y[:, c * ROWS:(c + 1) * ROWS, :] = results[c]["out"]
    return y


def run(inputs, trace=False, trace_cores=None):
    nc = _get_program()
    in_maps = make_in_maps(**inputs)
    res = run_bass_kernel_spmd(
        nc, in_maps, list(range(NC_)), trace=trace,
        trace_cores=trace_cores if trace else None)
    return assemble(res.results), res


def kernel(**inputs) -> np.ndarray:
    y, _ = run(inputs, trace=False)
    return y


# revision 9
# speedup vs baseline: 1.3322x; 1.0374x over previous
"""Causal self-attention (B=2, T=2048, D=2048, H=16, d=128) on 8 TRN2 NeuronCores.

Sharding: head-parallel compute, token-parallel output. Core c owns heads
{2c, 2c+1} for both batches: column-parallel QKV projection, per-head RoPE +
causal attention. The per-head attention outputs are exchanged with one
AllToAll per (batch, head), after which every core holds all 16 heads for its
own 256 rows and computes the full output projection locally. Host
concatenates the 8 contiguous row shards.

Schedule (v2): QKV projections for BOTH batches run before any collective so
launch skew between cores is absorbed by local compute, and no local DMA ever
queues behind collective descriptors in the HW DMA rings (post-collective
reads/writes go through gpsimd software DMA instead). The PE runs only the
essential matmuls: causal masking is a 0/1 multiply on DVE after the exp, and
softmax denominators are accumulated on DVE with a single f32r ones-matmul
per (head, 512-query) tile. Exp is applied to fused [128, 1024] score pairs
to halve ACT instruction overhead.

Host-prepped layouts (sharding/layout prep only — all math on device):
  xT      [2, D, T]    x transposed per batch (bf16)
  wqk     [D, 512]     qkv_w rows [q_h0,q_h1,k_h0,k_h1] transposed (bf16)
  wv      [D, 256]     qkv_w v rows transposed (bf16)
  wo      [D, D]       full out_w transposed (bf16)
  cosT/sinTs [128, T]  RoPE tables transposed; sinTs rows 0:64 negated
  masks   [4, 128, 512] multiplicative causal masks (1 / 0) for diag blocks
Matmuls run bf16 (1cyc/row); accumulation fp32 in PSUM; softmax
denominators fp32.
"""
import math
import numpy as np
import ml_dtypes
from contextlib import ExitStack

import concourse.bass as bass
import concourse.tile as tile
from concourse import bacc, mybir
from concourse.bass_utils import run_bass_kernel_spmd

F32 = mybir.dt.float32
F32R = mybir.dt.float32r
BF16 = mybir.dt.bfloat16
BF16_NP = ml_dtypes.bfloat16
AF = mybir.ActivationFunctionType
ALU = mybir.AluOpType

NC_ = 8           # cores
B, T, D = 2, 2048, 2048
H, HD = 16, 128   # heads, head_dim
HPC = H // NC_    # heads per core = 2
TS = 512          # t-super tile
NTS = T // TS     # 4
NCH = D // 128    # 16 contraction chunks
ROWS = T // NC_   # 256 own token rows per batch
SCALE = 1.0 / math.sqrt(HD)


def _build_program():
    nc = bacc.Bacc("TRN2", target_bir_lowering=False, debug=False, num_devices=NC_)

    xT_d = nc.dram_tensor("xT", [B, D, T], BF16, kind="ExternalInput")
    wqk_d = nc.dram_tensor("wqk", [D, 4 * 128], BF16, kind="ExternalInput")
    wv_d = nc.dram_tensor("wv", [D, 2 * 128], BF16, kind="ExternalInput")
    wo_d = nc.dram_tensor("wo", [D, D], BF16, kind="ExternalInput")
    cos_d = nc.dram_tensor("cosT", [128, T], BF16, kind="ExternalInput")
    sin_d = nc.dram_tensor("sinTs", [128, T], BF16, kind="ExternalInput")
    mask_d = nc.dram_tensor("masks", [4, 128, TS], BF16, kind="ExternalInput")
    ones_d = nc.dram_tensor("ones", [128, 128], F32, kind="ExternalInput")
    bqk_d = nc.dram_tensor("bqk", [128, 4], F32, kind="ExternalInput")
    bv_d = nc.dram_tensor("bv", [1, 2 * 128], BF16, kind="ExternalInput")
    bo_d = nc.dram_tensor("bo", [1, D], F32, kind="ExternalInput")
    out_d = nc.dram_tensor("out", [B, ROWS, D], F32, kind="ExternalOutput")

    with tile.TileContext(nc) as tc:
        with ExitStack() as ctx:
            consts = ctx.enter_context(tc.tile_pool(name="consts", bufs=1))
            qkv = ctx.enter_context(tc.tile_pool(name="qkv", bufs=1))
            dramp = ctx.enter_context(tc.tile_pool(name="dramp", bufs=1, space="DRAM"))

            wo_r = wo_d.ap().rearrange("(h p) o -> p h o", p=128)
            wo_t = consts.tile([128, H, D], BF16)
            cos_t = consts.tile([128, T], BF16)
            nc.scalar.dma_start(out=cos_t, in_=cos_d.ap())
            sin_t = consts.tile([128, T], BF16)
            nc.scalar.dma_start(out=sin_t, in_=sin_d.ap())
            mask_t = consts.tile([128, 4, TS], BF16)
            nc.scalar.dma_start(out=mask_t, in_=mask_d.ap().rearrange("m p n -> p m n"))
            ones_t = consts.tile([128, 128], F32)
            nc.scalar.dma_start(out=ones_t, in_=ones_d.ap())
            ones_b = consts.tile([128, 128], BF16)
            nc.scalar.activation(ones_b[:], ones_t[:], AF.Copy)
            bqk_t = consts.tile([128, 4], F32)
            nc.scalar.dma_start(out=bqk_t, in_=bqk_d.ap())
            bv_t = consts.tile([128, 2 * 128], BF16)
            nc.gpsimd.dma_start(out=bv_t, in_=bv_d.ap().partition_broadcast(128))
            bo_t = consts.tile([128, D], F32)
            nc.gpsimd.dma_start(out=bo_t, in_=bo_d.ap().partition_broadcast(128))

            q_t = {b: qkv.tile([128, HPC, T], BF16, tag=f"q{b}", name=f"q_t{b}")
                   for b in range(B)}
            k_t = {b: qkv.tile([128, HPC, T], BF16, tag=f"k{b}", name=f"k_t{b}")
                   for b in range(B)}
            v_t = {b: qkv.tile([128, NTS * 4, HPC, 128], BF16, tag=f"v{b}",
                               name=f"v_t{b}") for b in range(B)}

            def stage1(b, s1w, wqk_t, wv_t, wqk_r, wv_r, xp, qep, tmp, s1ps):
                for ts in range(NTS):
                    qkp = [s1ps.tile([128, TS], F32, tag=f"qkp{j}", name=f"qkp{j}")
                           for j in range(4)]
                    vp = [s1ps.tile([128, 2 * 128], F32, tag=f"vp{tb}",
                                    name=f"vp{tb}")[:] for tb in range(4)]
                    for ci in range(NCH):
                        if b == 0 and ts == 0:
                            nc.sync.dma_start(out=wqk_t[:, ci, :], in_=wqk_r[:, ci, :])
                            nc.sync.dma_start(out=wv_t[:, ci, :], in_=wv_r[:, ci, :])
                        elif b == 0 and ts == 1:
                            nc.sync.dma_start(out=wo_t[:, ci, :], in_=wo_r[:, ci, :])
                        xt = xp.tile([128, TS], BF16)
                        nc.sync.dma_start(
                            out=xt,
                            in_=xT_d.ap()[b, ci * 128:(ci + 1) * 128,
                                          ts * TS:(ts + 1) * TS],
                        )
                        st_, sp_ = ci == 0, ci == NCH - 1
                        for j in range(4):
                            nc.tensor.matmul(
                                qkp[j][:], wqk_t[:, ci, j * 128:(j + 1) * 128], xt[:],
                                start=st_, stop=sp_)
                        for tb in range(4):
                            nc.tensor.matmul(
                                vp[tb], xt[:, tb * 128:(tb + 1) * 128],
                                wv_t[:, ci, :], start=st_, stop=sp_)
                    # evict q/k to bf16 on ACT (plus a half-swapped copy for
                    # rotate_half); RoPE + bias fused on DVE. sinTs rows 0:64
                    # carry the rotate_half sign flip.
                    cs = cos_t[:, ts * TS:(ts + 1) * TS]
                    sn = sin_t[:, ts * TS:(ts + 1) * TS]
                    last_tile = (b == B - 1 and ts == NTS - 1)
                    if last_tile:
                        for tb in range(4):
                            vdst = v_t[b][:, ts * 4 + tb, :, :]
                            nc.scalar.activation(
                                vdst, vp[tb].rearrange("p (h e) -> p h e", h=HPC),
                                AF.Copy)
                            nc.vector.tensor_add(
                                vdst, vdst,
                                bv_t[:].rearrange("p (h e) -> p h e", h=HPC))
                    for j in range(4):
                        qe = qep.tile([128, TS], BF16, tag=f"qe{j}", name=f"qe{j}",
                                      bufs=2)
                        qs = qep.tile([128, TS], BF16, tag=f"qs{j}", name=f"qs{j}",
                                      bufs=2)
                        nc.scalar.activation(qe[:], qkp[j][:], AF.Copy)
                        nc.scalar.activation(qs[0:64, :], qe[64:128, :], AF.Copy)
                        nc.scalar.activation(qs[64:128, :], qe[0:64, :], AF.Copy)
                        t1 = tmp.tile([128, TS], BF16, tag="t1", bufs=2)
                        t2 = tmp.tile([128, TS], BF16, tag="t2", bufs=2)
                        nc.vector.tensor_mul(t1[:], qe[:], cs)
                        nc.vector.tensor_mul(t2[:], qs[:], sn)
                        dst = (q_t[b] if j < 2 else k_t[b])[:, j % 2,
                                                            ts * TS:(ts + 1) * TS]
                        nc.vector.scalar_tensor_tensor(
                            dst, t1[:], bqk_t[:, j:j + 1], t2[:], ALU.add, ALU.add)
                    if not last_tile:
                        for tb in range(4):
                            vdst = v_t[b][:, ts * 4 + tb, :, :]
                            nc.scalar.activation(
                                vdst, vp[tb].rearrange("p (h e) -> p h e", h=HPC),
                                AF.Copy)
                            nc.vector.tensor_add(
                                vdst, vdst,
                                bv_t[:].rearrange("p (h e) -> p h e", h=HPC))

            def attention(b, atps, prp, accp, bsp, aosp):
                # one AllToAll per (b, head); triggered as soon as that head's
                # normalized outputs are in DRAM. The per-tile epilogue
                # (denominator matmul, reciprocal, normalize, DRAM write) is
                # deferred until the next tile's first score pair so the PE
                # never waits on the DVE/gpsimd accumulation chain.
                a2a_in = [dramp.tile([NC_, 128, ROWS], BF16, tag=f"a2i{b}{hh}",
                                     name=f"a2i{b}{hh}") for hh in range(HPC)]
                a2a_out = [dramp.tile([NC_, 128, ROWS], BF16, tag=f"a2o{b}{hh}",
                                      name=f"a2o{b}{hh}") for hh in range(HPC)]

                def epilogue(pend):
                    op, acc, hh, ts = pend
                    sm = atps.tile([128, TS], F32, tag="sm", bufs=1)
                    nc.tensor.matmul(sm[:], ones_b[:], acc[:], start=True,
                                     stop=True)
                    bsb = bsp.tile([128, TS], F32, tag="bsb", bufs=2)
                    with nc.allow_low_precision(reason="softmax recip"):
                        nc.vector.reciprocal_approx_fast(bsb[:], sm[:])
                    aos = aosp.tile([128, TS], BF16, tag="aos", bufs=3)
                    nc.vector.tensor_mul(aos[:], op[:], bsb[:])
                    nc.gpsimd.dma_start(
                        out=a2a_in[hh][2 * ts:2 * ts + 2, :, :].transpose(
                            [1, 0, 2]),
                        in_=aos[:].rearrange("d (s q) -> d s q", s=2))

                pend = None
                for hh in range(HPC):
                    for ts in range(NTS):
                        op = None
                        acc = accp.tile([128, TS], BF16, tag="acc", bufs=2)
                        npair = 2 * (ts + 1)
                        prev = None
                        for p in range(npair):
                            st = atps.tile([128, 2, TS], F32, tag="st", bufs=2)
                            # diagonal pairs (ts>0): queries below the block
                            # row are fully masked; skip their columns. The
                            # full-width mask multiply zeroes whatever the
                            # skipped region holds.
                            diag = p >= 2 * ts
                            q0 = 256 if (ts > 0 and diag and p == 2 * ts + 1) else 0
                            for h2 in range(2):
                                tk = 2 * p + h2
                                nc.tensor.matmul(
                                    st[:, h2, q0:],
                                    k_t[b][:, hh, tk * 128:(tk + 1) * 128],
                                    q_t[b][:, hh,
                                          ts * TS + q0:(ts + 1) * TS],
                                    start=True, stop=True)
                            if op is None:
                                op = atps.tile([128, TS], F32, tag="op", bufs=2)
                            if p == 1 and pend is not None:
                                epilogue(pend)
                                pend = None
                            pr = prp.tile([128, 2, TS], BF16, tag="pr", bufs=3)
                            nc.scalar.activation(pr[:, :, q0:], st[:, :, q0:],
                                                 AF.Exp, scale=SCALE)
                            if diag:  # zero masked scores (and skipped cols)
                                mi = p - 2 * ts
                                nc.vector.tensor_mul(
                                    pr[:], pr[:], mask_t[:, 2 * mi:2 * mi + 2, :])
                            ps = bsp.tile([128, TS], BF16, tag="ps", bufs=2)
                            nc.vector.tensor_add(ps[:], pr[:, 0, :], pr[:, 1, :])
                            if p == 0:
                                nc.vector.tensor_scalar_add(acc[:], ps[:], 0.0)
                            else:
                                nc.vector.tensor_add(acc[:], acc[:], ps[:])
                            if prev is not None:
                                pp, ppr = prev
                                for h2 in range(2):
                                    tkl = 2 * pp + h2 - 4 * ts
                                    a0 = 128 * tkl if (ts > 0 and tkl >= 0) else 0
                                    nc.tensor.matmul(
                                        op[:, a0:], v_t[b][:, 2 * pp + h2, hh, :],
                                        ppr[:, h2, a0:],
                                        start=(pp == 0 and h2 == 0), stop=False)
                            prev = (p, pr)
                        pp, ppr = prev
                        for h2 in range(2):
                            tkl = 2 * pp + h2 - 4 * ts
                            a0 = 128 * tkl if (ts > 0 and tkl >= 0) else 0
                            nc.tensor.matmul(
                                op[:, a0:], v_t[b][:, 2 * pp + h2, hh, :],
                                ppr[:, h2, a0:],
                                start=(pp == 0 and h2 == 0), stop=(h2 == 1))
                        pend = (op, acc, hh, ts)
                    # flush before the collective: it needs every tile's aos
                    epilogue(pend)
                    pend = None
                    nc.gpsimd.collective_compute(
                        "AllToAll", mybir.AluOpType.bypass,
                        replica_groups=[list(range(NC_))],
                        ins=[a2a_in[hh].opt()], outs=[a2a_out[hh].opt()])
                return a2a_out

            def outproj(b, a2a_out, aogp, yop, yps):
                # a2a_out[hh][src, d, q] == head (2*src+hh) for my ROWS of batch b
                aoG = [aogp.tile([128, NC_, ROWS], BF16, tag=f"aoG{hh}",
                                 name=f"aoG{hh}") for hh in range(HPC)]
                for hh in range(HPC):
                    nc.sync.dma_start(
                        out=aoG[hh],
                        in_=a2a_out[hh][:, :, :].rearrange("s d q -> d s q"))
                # tile-at-a-time: each (tb, nb) chunk accumulates its 16
                # head contributions back-to-back, then evacuates while the
                # next chunk computes — output writes pipeline instead of
                # bursting at the end.
                for tb in range(2):
                    for nb in range(4):
                        yp = yps.tile([128, TS], F32, tag="yp", bufs=4)
                        for hh in range(HPC):
                            for s in range(NC_):
                                nc.tensor.matmul(
                                    yp[:], aoG[hh][:, s, tb * 128:(tb + 1) * 128],
                                    wo_t[:, 2 * s + hh, nb * TS:(nb + 1) * TS],
                                    start=(hh == 0 and s == 0),
                                    stop=(hh == 1 and s == NC_ - 1))
                        yo = yop.tile([128, TS], F32, tag="yo", bufs=3)
                        nc.vector.tensor_add(yo[:], yp[:],
                                             bo_t[:, nb * TS:(nb + 1) * TS])
                        nc.scalar.dma_start(
                            out=out_d.ap()[b, tb * 128:(tb + 1) * 128,
                                           nb * TS:(nb + 1) * TS],
                            in_=yo[:])

            # ---- phase 1: QKV projections for both batches (no collectives) --
            with tc.tile_pool(name="s1w", bufs=1) as s1w, \
                    tc.tile_pool(name="xp", bufs=12) as xp, \
                    tc.tile_pool(name="qep", bufs=1) as qep, \
                    tc.tile_pool(name="tmp", bufs=1) as tmp, \
                    tc.tile_pool(name="s1ps", bufs=1, space="PSUM") as s1ps:
                wqk_t = s1w.tile([128, NCH, 4 * 128], BF16)
                wv_t = s1w.tile([128, NCH, 2 * 128], BF16)
                wqk_r = wqk_d.ap().rearrange("(c p) e -> p c e", p=128)
                wv_r = wv_d.ap().rearrange("(c p) e -> p c e", p=128)
                for b in range(B):
                    stage1(b, s1w, wqk_t, wv_t, wqk_r, wv_r, xp, qep, tmp, s1ps)

            # ---- phase 2: attention + exchanges --------------------------
            a2a_outs = {}
            with tc.tile_pool(name="atps", bufs=1, space="PSUM") as atps, \
                    tc.tile_pool(name="prp", bufs=1) as prp, \
                    tc.tile_pool(name="accp", bufs=1) as accp, \
                    tc.tile_pool(name="bsp", bufs=1) as bsp, \
                    tc.tile_pool(name="aosp", bufs=1) as aosp:
                for b in range(B):
                    a2a_outs[b] = attention(b, atps, prp, accp, bsp, aosp)

            # ---- phase 3: output projections -----------------------------
            with tc.tile_pool(name="yps", bufs=1, space="PSUM") as yps, \
                    tc.tile_pool(name="aogp", bufs=1) as aogp, \
                    tc.tile_pool(name="yop", bufs=1) as yop:
                for b in range(B):
                    outproj(b, a2a_outs[b], aogp, yop, yps)

    nc.compile()
    return nc


_NC_CACHE = None


def _get_program():
    global _NC_CACHE
    if _NC_CACHE is None:
        _NC_CACHE = _build_program()
    return _NC_CACHE


def make_in_maps(x, rope_cos, rope_sin, qkv_w, qkv_b, out_w, out_b):
    x = np.asarray(x, dtype=np.float32)
    qkv_w = np.asarray(qkv_w, dtype=np.float32)
    qkv_b = np.asarray(qkv_b, dtype=np.float32)
    out_w = np.asarray(out_w, dtype=np.float32)
    out_b = np.asarray(out_b, dtype=np.float32)

    xT = np.ascontiguousarray(x.transpose(0, 2, 1)).astype(BF16_NP)  # [B, D, T]
    cosT = np.ascontiguousarray(np.asarray(rope_cos, np.float32)[0, 0].T).astype(BF16_NP)
    sinTs = np.ascontiguousarray(np.asarray(rope_sin, np.float32)[0, 0].T).copy()
    sinTs[0:64, :] *= -1.0
    sinTs = sinTs.astype(BF16_NP)

    tk_idx = np.arange(128)[:, None]
    tq_idx = np.arange(TS)[None, :]
    masks = np.stack(
        [np.where(mi * 128 + tk_idx <= tq_idx, 1.0, 0.0) for mi in range(4)]
    ).astype(BF16_NP)                                           # [4, 128, TS]
    ones = np.ones((128, 128), np.float32)
    wo = np.ascontiguousarray(out_w.T).astype(BF16_NP)          # [D, D]
    bo = out_b.reshape(1, D)

    in_maps = []
    for c in range(NC_):
        h0 = HPC * c
        qr = qkv_w[h0 * 128:(h0 + HPC) * 128]                  # [256, D]
        kr = qkv_w[D + h0 * 128:D + (h0 + HPC) * 128]
        vr = qkv_w[2 * D + h0 * 128:2 * D + (h0 + HPC) * 128]
        wqk = np.ascontiguousarray(np.concatenate([qr, kr], 0).T).astype(BF16_NP)
        wv = np.ascontiguousarray(vr.T).astype(BF16_NP)        # [D, 256]
        bqk = np.stack(
            [qkv_b[h0 * 128:(h0 + 1) * 128],
             qkv_b[(h0 + 1) * 128:(h0 + 2) * 128],
             qkv_b[D + h0 * 128:D + (h0 + 1) * 128],
             qkv_b[D + (h0 + 1) * 128:D + (h0 + 2) * 128]], axis=1)  # [128, 4]
        bv = qkv_b[2 * D + h0 * 128:2 * D + (h0 + HPC) * 128].reshape(1, 256)
        in_maps.append({
            "xT": xT, "wqk": wqk, "wv": wv, "wo": wo,
            "cosT": cosT, "sinTs": sinTs, "masks": masks, "ones": ones,
            "bqk": np.ascontiguousarray(bqk),
            "bv": np.ascontiguousarray(bv).astype(BF16_NP),
            "bo": bo,
        })
    return in_maps


def assemble(results):
    y = np.empty((B, T, D), dtype=np.float32)
    for c in range(NC_):
        y[:, c * ROWS:(c + 1) * ROWS, :] = results[c]["out"]
    return y


def run(inputs, trace=False, trace_cores=None):
    nc = _get_program()
    in_maps = make_in_maps(**inputs)
    res = run_bass_kernel_spmd(
        nc, in_maps, list(range(NC_)), trace=trace,
        trace_cores=trace_cores if trace else None)
    return assemble(res.results), res


def kernel(**inputs) -> np.ndarray:
    y, _ = run(inputs, trace=False)
    return y
